# revision 4
# baseline (speedup 1.0000x reference)
"""Multi-head attention (B=2, S=2048, D=1024, H=16) on 8 TRN2 NeuronCores, v2.

Sharding: data-parallel over batch (2) x tensor-parallel over head groups (4).
Core c handles batch c//4, heads [4*(c%4), 4*(c%4)+4).  Host sums the 8
partial outputs (the Wo row-slice all-reduce) and adds bo.

v2 vs v1: attention runs in 64x64 PE-tile mode so the systolic array is
fully packed despite head_dim=64:
  - scores^T: 4 concurrent quadrant matmuls (2 heads x 2 k-halves), no
    zero-padded contraction rows
  - PV: 4 concurrent quadrants (2 heads x 2 contraction halves), partials
    folded by DVE
  - softmax denominators: all-ones [64,64] stationary on the same 4
    quadrants -> denominators come out pre-broadcast over 64 rows, fp32
Causal mode additionally restricts scores/exp/PV/den to the valid column
window of diagonal tiles.  Phases: dense 128-mode QKV projections first,
then attention, with Wo chunks batched at q-chunk boundaries.
"""

import numpy as np

import concourse.bass as bass
import concourse.mybir as mybir
from concourse import bacc
from concourse.tile import TileContext
from concourse.bass_utils import run_bass_kernel_spmd

P = 128
S = 2048
D = 1024
H = 16
HD = 64
B = 2
NCORES = 8
HGROUPS = 4
HC = H // HGROUPS          # 4 heads per core
DC = HC * HD               # 256-wide weight slice per core
NST = S // P               # 16 s-tiles (= k-tiles inside attention)
NKT = D // P               # 8 contraction tiles for the projections
QCW = 512
NQC = S // QCW             # 4 q-chunks

F32 = mybir.dt.float32
F16 = mybir.dt.float16
AF = mybir.ActivationFunctionType
OP = mybir.AluOpType
EXP_BIAS = -12.0           # keeps exp(q.k/8 - 12) inside fp16 range


def _build(mode, zero_bias):
    """mode: 'causal' | 'allones' | 'general'."""
    nc = bacc.Bacc("TRN2", debug=False, num_devices=NCORES,
                   num_swdge_queues=4)

    xt_in = nc.dram_tensor("xt", [P, NKT, S], F16, kind="ExternalInput")
    wq = nc.dram_tensor("wq", [D, DC], F16, kind="ExternalInput")
    wk = nc.dram_tensor("wk", [D, DC], F16, kind="ExternalInput")
    wv = nc.dram_tensor("wv", [D, DC], F16, kind="ExternalInput")
    wo = nc.dram_tensor("wo", [DC, D], F16, kind="ExternalInput")
    if not zero_bias:
        bq = nc.dram_tensor("bq", [DC], F32, kind="ExternalInput")
        bk = nc.dram_tensor("bk", [DC], F32, kind="ExternalInput")
        bv = nc.dram_tensor("bv", [DC], F32, kind="ExternalInput")
    mmast = None
    maskt = None
    if mode == "causal":
        mmast = nc.dram_tensor("mmast", [P, 512], F16, kind="ExternalInput")
    elif mode == "general":
        maskt = nc.dram_tensor("maskt", [NST, P, S], F16, kind="ExternalInput")
    out = nc.dram_tensor("out", [S, D], F16, kind="ExternalOutput")

    def nvalid_of(qc):
        return 4 * (qc + 1) if mode == "causal" else NST

    def w0_of(qc, kt):
        if mode == "causal" and kt >= 4 * qc:
            return P * (kt - 4 * qc)
        return 0

    with TileContext(nc) as tc:
        with tc.tile_pool(name="big", bufs=1) as big:
            warm = big.tile([1, QCW], F16, tag="warm", name="warm")
            nc.gpsimd.memset(warm[:], 1.0)
            # ---------- constants ----------
            ones = big.tile([P, HD], F16, tag="ones", name="ones")
            nc.vector.memset(ones[:], 1.0)
            ebias = big.tile([P, 1], F32, tag="ebias", name="ebias")
            nc.vector.memset(ebias[:], EXP_BIAS)
            ones16 = big.tile([1, P], F16, tag="ones16", name="ones16")
            nc.vector.memset(ones16[:], 1.0)
            if not zero_bias:
                bq32 = big.tile([P, 2], F32, tag="bq32", name="bq32")
                bk32 = big.tile([P, 2], F32, tag="bk32", name="bk32")
                bv32 = big.tile([1, DC], F32, tag="bv32", name="bv32")
                bv16 = big.tile([1, DC], F16, tag="bv16", name="bv16")
            mast16 = None
            if mode == "causal":
                mast16 = big.tile([P, 512], F16, tag="mast16", name="mast16")

            # ---------- persistent fp16 operands ----------
            xT = big.tile([P, NKT, S], F16, tag="xT", name="xT")
            QT = [big.tile([P, S], F16, tag=f"QT{m}", name=f"QT{m}")
                  for m in range(2)]
            KT = [big.tile([P, S], F16, tag=f"KT{m}", name=f"KT{m}")
                  for m in range(2)]
            V = [big.tile([P, HC, HD], F16, tag=f"V{st}", name=f"V{st}")
                 for st in range(NST)]
            outT = [big.tile([P, S], F16, tag=f"outT{m}", name=f"outT{m}")
                    for m in range(2)]
            wq16 = big.tile([P, NKT, DC], F16, tag="wq16", name="wq16")
            wk16 = big.tile([P, NKT, DC], F16, tag="wk16", name="wk16")
            wv16 = big.tile([P, NKT, DC], F16, tag="wv16", name="wv16")
            wo16 = big.tile([P, 2, D], F16, tag="wo16", name="wo16")

            with nc.named_scope("prep"):
                # load the Exp table while DMAs run
                wact = big.tile([1, QCW], F16, tag="wact", name="wact")
                nc.scalar.activation(wact[:], warm[:], AF.Exp,
                                     bias=ebias[0:1, :], scale=0.125)
                wqr = wq.ap().rearrange("(t p) c -> p t c", p=P)
                wkr = wk.ap().rearrange("(t p) c -> p t c", p=P)
                wvr = wv.ap().rearrange("(t p) c -> p t c", p=P)
                wor = wo.ap().rearrange("(t p) c -> p t c", p=P)
                h0, h1 = slice(0, 4), slice(4, 8)
                HQ = QCW // 2
                nc.sync.dma_start(xT[:, :, 0:HQ], xt_in[:, :, 0:HQ])
                nc.scalar.dma_start(xT[:, :, HQ:QCW], xt_in[:, :, HQ:QCW])
                nc.gpsimd.dma_start(wq16[:, h0, :], wqr[:, h0, :])
                nc.gpsimd.dma_start(wq16[:, h1, :], wqr[:, h1, :])
                nc.gpsimd.dma_start(wk16[:, h0, :], wkr[:, h0, :])
                nc.gpsimd.dma_start(wk16[:, h1, :], wkr[:, h1, :])
                nc.sync.dma_start(xT[:, :, QCW:QCW + HQ],
                                  xt_in[:, :, QCW:QCW + HQ])
                nc.scalar.dma_start(xT[:, :, QCW + HQ:2 * QCW],
                                    xt_in[:, :, QCW + HQ:2 * QCW])
                nc.gpsimd.dma_start(wv16[:, h0, :], wvr[:, h0, :])
                nc.gpsimd.dma_start(wv16[:, h1, :], wvr[:, h1, :])
                if not zero_bias:
                    nc.gpsimd.dma_start(bq32[:],
                                        bq.ap().rearrange("(o p) -> p o", p=P))
                    nc.gpsimd.dma_start(bk32[:],
                                        bk.ap().rearrange("(o p) -> p o", p=P))
                    nc.gpsimd.dma_start(bv32[:], bv.ap().unsqueeze(0))
                    nc.vector.tensor_copy(bv16[:], bv32[:])
                if mode == "causal":
                    nc.gpsimd.dma_start(mast16[:], mmast[:])
                nc.sync.dma_start(xT[:, :, 2 * QCW:3 * QCW],
                                  xt_in[:, :, 2 * QCW:3 * QCW])
                nc.gpsimd.dma_start(xT[:, :, 3 * QCW:S],
                                    xt_in[:, :, 3 * QCW:S])
                nc.gpsimd.dma_start(wo16[:, 0, :], wor[:, 0, :])
                nc.gpsimd.dma_start(wo16[:, 1, :], wor[:, 1, :])

            # ---------- attention + interleaved projection fillers ---------
            # scores, proj-chain fragments and Wo chunks all draw transient
            # PSUM from the same pool (scp); pvp/dnp hold only the per-qc
            # accumulators, so filler pops can never wrap a live accumulator
            with nc.named_scope("main"), \
                 tc.tile_pool(name="sc", bufs=2, space="PSUM") as scp, \
                 tc.tile_pool(name="pv", bufs=2, space="PSUM") as pvp, \
                 tc.tile_pool(name="dn", bufs=2, space="PSUM") as dnp, \
                 tc.tile_pool(name="p16", bufs=8) as p16p, \
                 tc.tile_pool(name="nrm", bufs=4) as nrm, \
                 tc.tile_pool(name="qsb", bufs=4) as qsb, \
                 tc.tile_pool(name="ost", bufs=3) as ost, \
                 tc.tile_pool(name="mt", bufs=1) as mtp:

                maskt_sb = {}
                if mode == "general":
                    for kt in range(NST):
                        mts = mtp.tile([P, S], F16, tag=f"mts{kt}",
                                       name=f"mts{kt}")
                        eng = (nc.sync, nc.scalar, nc.gpsimd)[kt % 3]
                        eng.dma_start(mts[:], maskt[kt])
                        maskt_sb[kt] = mts

                def qk_one(mb, qc, wi, warmups=0):
                    qs = slice(qc * QCW, (qc + 1) * QCW)
                    w16 = (wq16, wk16)[wi]
                    ps = scp.tile([P, 2, QCW], F32, tag="sx", name="chain")
                    for w in range(warmups):
                        nc.tensor.matmul(ps[:, 0, :], warm[:, 0:P], warm[:],
                                         start=True, stop=True)
                    for kt in range(NKT):
                        nc.tensor.matmul(
                            ps[:, 0, :], w16[:, kt, mb * P:(mb + 1) * P],
                            xT[:, kt, qs],
                            start=(kt == 0), stop=(kt == NKT - 1))
                    dst = (QT, KT)[wi][mb]
                    if zero_bias:
                        nc.vector.tensor_copy(dst[:, qs], ps[:, 0, :])
                    else:
                        bcol = (bq32, bk32)[wi]
                        nc.vector.tensor_scalar_add(
                            dst[:, qs], ps[:, 0, :], bcol[:, mb:mb + 1])

                def v_chain(st):
                    ps = scp.tile([P, 2, QCW], F32, tag="sx", name="chain")
                    pv = ps[:, 0, 0:DC]
                    for kt in range(NKT):
                        nc.tensor.matmul(
                            pv, xT[:, kt, st * P:(st + 1) * P], wv16[:, kt, :],
                            start=(kt == 0),
                            stop=(kt == NKT - 1 and zero_bias))
                    if not zero_bias:
                        nc.tensor.matmul(pv, ones16[:], bv16[:],
                                         start=False, stop=True)
                    nc.vector.tensor_copy(
                        V[st][:],
                        ps[:, 0, 0:DC].rearrange("p (h d) -> p h d", h=HC))

                pending = []

                def attention(hp, qc):
                    nvalid = nvalid_of(qc)
                    q0 = qc * QCW
                    box = {}

                    def scores_exp(kt):
                        w0 = w0_of(qc, kt)
                        ka = slice(kt * P, kt * P + HD)
                        kb = slice(kt * P + HD, (kt + 1) * P)
                        qw = slice(q0 + w0, q0 + QCW)
                        sx = scp.tile([P, 2, QCW], F32, tag="sx", name="sx")
                        nc.tensor.matmul(sx[0:64, 0, w0:QCW],
                                         KT[hp][0:64, ka], QT[hp][0:64, qw],
                                         start=True, stop=True)
                        nc.tensor.matmul(sx[64:128, 0, w0:QCW],
                                         KT[hp][0:64, kb], QT[hp][0:64, qw],
                                         start=True, stop=True)
                        nc.tensor.matmul(sx[0:64, 1, w0:QCW],
                                         KT[hp][64:128, ka],
                                         QT[hp][64:128, qw],
                                         start=True, stop=True)
                        nc.tensor.matmul(sx[64:128, 1, w0:QCW],
                                         KT[hp][64:128, kb],
                                         QT[hp][64:128, qw],
                                         start=True, stop=True)
                        p16 = p16p.tile([P, 2, QCW], F16, tag="p16",
                                        name="p16")
                        if w0 == 0:
                            nc.scalar.activation(
                                p16[:].rearrange("p a b -> p (a b)"),
                                sx[:].rearrange("p a b -> p (a b)"),
                                AF.Exp, bias=ebias[:], scale=0.125)
                        else:
                            nc.scalar.activation(p16[:, :, w0:QCW],
                                                 sx[:, :, w0:QCW], AF.Exp,
                                                 bias=ebias[:], scale=0.125)
                        if mode == "causal" and kt >= 4 * qc:
                            pb = p16[:, :, w0:w0 + P]
                            nc.vector.tensor_tensor(
                                pb, pb,
                                mast16[:, 0:P].unsqueeze(1).to_broadcast(
                                    (P, 2, P)),
                                OP.mult)
                        elif mode == "general":
                            pv2 = p16[:, :, 0:QCW]
                            nc.vector.tensor_tensor(
                                pv2, pv2,
                                maskt_sb[kt][:, q0:q0 + QCW].unsqueeze(1)
                                .to_broadcast((P, 2, QCW)),
                                OP.mult)
                        return p16

                    def pv_den(kt, p16):
                        pvP, pvQ = box["pvP"], box["pvQ"]
                        dnA, dnB = box["dnA"], box["dnB"]
                        w0 = w0_of(qc, kt)
                        first = kt == 0
                        last = kt == nvalid - 1
                        hA, hB = 2 * hp, 2 * hp + 1
                        # PV: 4 concurrent quadrants (2 heads x 2 k-halves)
                        nc.tensor.matmul(pvP[0:64, w0:QCW],
                                         V[kt][0:64, hA, :],
                                         p16[0:64, 0, w0:QCW],
                                         start=first, stop=last,
                                         skip_group_check=True)
                        nc.tensor.matmul(pvP[64:128, w0:QCW],
                                         V[kt][0:64, hB, :],
                                         p16[0:64, 1, w0:QCW],
                                         start=first, stop=last,
                                         skip_group_check=True)
                        nc.tensor.matmul(pvQ[0:64, w0:QCW],
                                         V[kt][64:128, hA, :],
                                         p16[64:128, 0, w0:QCW],
                                         start=first, stop=last,
                                         skip_group_check=True)
                        nc.tensor.matmul(pvQ[64:128, w0:QCW],
                                         V[kt][64:128, hB, :],
                                         p16[64:128, 1, w0:QCW],
                                         start=first, stop=last,
                                         skip_group_check=True)
                        # denominators: all-ones stationary, pre-broadcast
                        nc.tensor.matmul(dnA[0:64, w0:QCW], ones[0:64, :],
                                         p16[0:64, 0, w0:QCW],
                                         start=first, stop=last,
                                         skip_group_check=True)
                        nc.tensor.matmul(dnA[64:128, w0:QCW], ones[0:64, :],
                                         p16[0:64, 1, w0:QCW],
                                         start=first, stop=last,
                                         skip_group_check=True)
                        nc.tensor.matmul(dnB[0:64, w0:QCW], ones[64:128, :],
                                         p16[64:128, 0, w0:QCW],
                                         start=first, stop=last,
                                         skip_group_check=True)
                        nc.tensor.matmul(dnB[64:128, w0:QCW], ones[64:128, :],
                                         p16[64:128, 1, w0:QCW],
                                         start=first, stop=last,
                                         skip_group_check=True)

                    # software pipeline: scores/exp run one k-tile ahead of
                    # PV/den so the PE never waits on the exp it just fed
                    # scores/exp run one k-tile ahead of PV/den; filler
                    # pops land right before the next scores group so they
                    # absorb the wait for the exp that gates its psum slot
                    p16_prev = scores_exp(0)
                    box["pvP"] = pvp.tile([P, QCW], F32, tag="pv", name="pvP")
                    box["pvQ"] = pvp.tile([P, QCW], F32, tag="pv", name="pvQ")
                    box["dnA"] = dnp.tile([P, QCW], F32, tag="dn", name="dnA")
                    box["dnB"] = dnp.tile([P, QCW], F32, tag="dn", name="dnB")
                    for kt in range(1, nvalid):
                        p16_cur = scores_exp(kt)
                        pv_den(kt - 1, p16_prev)
                        p16_prev = p16_cur
                        if pending:
                            pending.pop(0)()
                    pv_den(nvalid - 1, p16_prev)
                    if pending:
                        pending.pop(0)()
                    pvP, pvQ = box["pvP"], box["pvQ"]
                    dnA, dnB = box["dnA"], box["dnB"]

                    # normalize: den = dnA + dnB (rows pre-broadcast), then
                    # outT = (pvP + pvQ) * recip(den)
                    qs = slice(q0, q0 + QCW)
                    pvQ_sb = qsb.tile([P, QCW], F32, tag="pvQ_sb",
                                      name="pvQ_sb")
                    nc.vector.tensor_copy(pvQ_sb[:], pvQ[:])
                    fold = nrm.tile([P, QCW], F32, tag="fold", name="fold")
                    nc.vector.tensor_tensor(fold[:], pvP[:], pvQ_sb[:],
                                            OP.add)
                    dnB_sb = qsb.tile([P, QCW], F32, tag="dnB_sb",
                                      name="dnB_sb")
                    nc.vector.tensor_copy(dnB_sb[:], dnB[:])
                    den = nrm.tile([P, QCW], F32, tag="den", name="den")
                    nc.vector.tensor_tensor(den[:], dnA[:], dnB_sb[:], OP.add)
                    rdb = nrm.tile([P, QCW], F32, tag="rdb", name="rdb")
                    nc.vector.reciprocal_approx_fast(rdb[:], den[:])
                    nc.vector.tensor_tensor(outT[hp][:, qs], fold[:], rdb[:],
                                            OP.mult)

                def d_chunk(qb, nh, pool=None):
                    ns = slice(nh * QCW, (nh + 1) * QCW)
                    if pool == "pv":
                        ps = pvp.tile([P, QCW], F32, tag="pv", name="dch")
                    elif pool == "dn":
                        ps = dnp.tile([P, QCW], F32, tag="dn", name="dch")
                    else:
                        ps3 = scp.tile([P, 2, QCW], F32, tag="sx", name="dch")
                        ps = ps3[:, 0, :]
                    for t in range(2):
                        nc.tensor.matmul(
                            ps[:], outT[t][:, qb * P:(qb + 1) * P],
                            wo16[:, t, ns], start=(t == 0), stop=(t == 1))
                    ob = ost.tile([P, QCW], F16, tag="ob", name="ob")
                    nc.vector.tensor_copy(ob[:], ps[:])
                    oeng = (nc.sync, nc.gpsimd)[(2 * qb + nh) % 2]
                    oeng.dma_start(out[qb * P:(qb + 1) * P, ns], ob[:])

                import functools

                # prefix: exactly what attention(0, 0) needs, PE-dense
                qk_one(0, 0, 0, warmups=12)
                qk_one(0, 0, 1)
                for st in range(4):
                    v_chain(st)

                # everything else runs as fillers popped between attention
                # iterations (keeps the PE dense and the HAM clock warm
                # while the ACT engine works through the exps)
                pending += [functools.partial(qk_one, 1, 0, 0),
                            functools.partial(qk_one, 1, 0, 1)]
                for qc in range(1, NQC):
                    for mb in range(2):
                        pending += [functools.partial(qk_one, mb, qc, 0),
                                    functools.partial(qk_one, mb, qc, 1)]
                    pending += [functools.partial(v_chain, st)
                                for st in range(4 * qc, 4 * qc + 4)]

                for qc in range(NQC):
                    attention(0, qc)
                    attention(1, qc)
                    for qb in range(4 * qc, 4 * qc + 4):
                        pending.append(functools.partial(d_chunk, qb, 0))
                        pending.append(functools.partial(d_chunk, qb, 1))
                drain_i = [0]
                while pending:
                    fn = pending.pop(0)
                    if getattr(fn, "func", None) is d_chunk:
                        fn(pool=(None, "pv", "dn")[drain_i[0] % 3])
                        drain_i[0] += 1
                    else:
                        fn()

    nc.compile()
    return nc


_BUILD_CACHE = {}


def _get_module(mode, zero_bias):
    key = (mode, zero_bias)
    if key not in _BUILD_CACHE:
        _BUILD_CACHE[key] = _build(mode, zero_bias)
    return _BUILD_CACHE[key]


def _causal_master():
    # mm[k, w] = 1 iff k <= w - 384, sliced as [:, 384:512] for the
    # boundary block of every diagonal tile
    kk = np.arange(P)[:, None]
    w = np.arange(384, 896)[None, :]
    return (kk <= w - 384).astype(np.float16)


def kernel(**inputs):
    x = np.ascontiguousarray(np.asarray(inputs["x"], dtype=np.float32))
    attn_mask = np.asarray(inputs["attn_mask"])
    Wq = np.asarray(inputs["Wq"], dtype=np.float32)
    Wk = np.asarray(inputs["Wk"], dtype=np.float32)
    Wv = np.asarray(inputs["Wv"], dtype=np.float32)
    Wo = np.asarray(inputs["Wo"], dtype=np.float32)
    bq = np.asarray(inputs["bq"], dtype=np.float32)
    bk = np.asarray(inputs["bk"], dtype=np.float32)
    bv = np.asarray(inputs["bv"], dtype=np.float32)
    bo = np.asarray(inputs["bo"], dtype=np.float32)

    m = attn_mask.reshape(B, attn_mask.shape[-2], attn_mask.shape[-1])
    if m.all():
        mode = "allones"
    elif all(np.array_equal(m[b], np.tril(np.ones((S, S), dtype=bool)))
             for b in range(B)):
        mode = "causal"
    else:
        mode = "general"
    zero_bias = not (bq.any() or bk.any() or bv.any())

    nc = _get_module(mode, zero_bias)

    in_maps = []
    for c in range(NCORES):
        b, hg = c // HGROUPS, c % HGROUPS
        cs = slice(hg * DC, (hg + 1) * DC)
        xt = x[b].T.astype(np.float16).reshape(NKT, P, S)
        im = {
            "xt": np.ascontiguousarray(xt.transpose(1, 0, 2)),
            "wq": np.ascontiguousarray(Wq[:, cs].astype(np.float16)),
            "wk": np.ascontiguousarray(Wk[:, cs].astype(np.float16)),
            "wv": np.ascontiguousarray(Wv[:, cs].astype(np.float16)),
            "wo": np.ascontiguousarray(Wo[cs, :].astype(np.float16)),
        }
        if not zero_bias:
            im["bq"] = np.ascontiguousarray(bq[cs])
            im["bk"] = np.ascontiguousarray(bk[cs])
            im["bv"] = np.ascontiguousarray(bv[cs])
        if mode == "causal":
            im["mmast"] = _causal_master()
        elif mode == "general":
            im["maskt"] = np.ascontiguousarray(
                m[b].T.astype(np.float16).reshape(NST, P, S))
        in_maps.append(im)

    res = run_bass_kernel_spmd(nc, in_maps, core_ids=list(range(NCORES)))

    out = np.zeros((B, S, D), dtype=np.float32)
    for c in range(NCORES):
        out[c // HGROUPS] += res.results[c]["out"].astype(np.float32)
    out += bo[None, None, :]
    return out
